# revision 16
# baseline (speedup 1.0000x reference)
"""Trainium2 Bass kernel for nn_CompressModel: y = FHT_1024(x * golay) / (alpha + eps).

Factorization: H_1024 = H_8 (outer, on feat bits 7-9) (x) H_128 (inner, feat bits 0-6).

Per-core dataflow (pure data-parallel over rows; 4096 rows/core, supertiles of 512 rows):
  1. DMA x supertile [512 rows, 1024] -> SBUF [128p, 4rg x 1024f]
  2. PE transposes each [128r, 128b] block -> PSUM zt_a [128b, 512(rg,r)]
  3. ScalarE drains PSUM->SBUF with per-partition scale g[a*128+b]/(alpha+eps)
     (golay multiply + normalization fused into the copy)
  4. VectorE: outer FHT_8 = 3 butterfly levels across the 8 a-blocks (transposed space)
  5. PE matmul out_blk[r,b'] = v_blk[b,r].T @ H128[b,b']  -- applies inner H_128 AND
     un-transposes in one op (lhsT = data stationary, rhs = H128 moving)
  6. ScalarE drains PSUM->SBUF, DMA out.
"""

import numpy as np
from contextlib import ExitStack

import concourse.bass as bass
import concourse.tile as tile
from concourse import bacc, mybir
from concourse.bass_utils import run_bass_kernel_spmd

f32 = mybir.dt.float32

N_CORES = 8
DIM = 1024
EPS = 1e-5
ROWS_TOTAL = 4 * 8192          # 32768
ROWS_PER_CORE = ROWS_TOTAL // N_CORES   # 4096
ST_ROWS = 256                  # rows per supertile
N_ST = ROWS_PER_CORE // ST_ROWS  # 16
RG = ST_ROWS // 128            # row-groups per supertile (2)
W = RG * 128                   # free-width per a-block in transposed space (256)

LAST_RESULT = None  # test harness reads exec_time_ns from here


def _hadamard(n: int) -> np.ndarray:
    h = np.array([[1.0]], dtype=np.float32)
    while h.shape[0] < n:
        h = np.block([[h, h], [h, -h]])
    return np.ascontiguousarray(h.astype(np.float32))


def _build_nc():
    nc = bacc.Bacc("TRN2", target_bir_lowering=False, debug=False)
    x_d = nc.dram_tensor("x", [ROWS_PER_CORE, DIM], f32, kind="ExternalInput")
    g_d = nc.dram_tensor("gvec", [128, 8], f32, kind="ExternalInput")
    h_d = nc.dram_tensor("hmat", [128, 128], f32, kind="ExternalInput")
    i_d = nc.dram_tensor("ident", [128, 128], f32, kind="ExternalInput")
    y_d = nc.dram_tensor("y", [ROWS_PER_CORE, DIM], f32, kind="ExternalOutput")

    with TileKernel(nc) as tk:
        tk.emit(x_d, g_d, h_d, i_d, y_d)

    nc.compile()
    return nc


class TileKernel:
    def __init__(self, nc):
        self.nc = nc
        self.ctx = ExitStack()

    def __enter__(self):
        self.tc = self.ctx.enter_context(tile.TileContext(self.nc))
        return self

    def __exit__(self, *exc):
        return self.ctx.__exit__(*exc)

    def emit(self, x_d, g_d, h_d, i_d, y_d):
        nc, tc, ctx = self.nc, self.tc, self.ctx

        const_pool = ctx.enter_context(tc.tile_pool(name="const", bufs=1))
        x_pool = ctx.enter_context(tc.tile_pool(name="x", bufs=4))
        u_pool = ctx.enter_context(tc.tile_pool(name="u", bufs=3))
        v_pool = ctx.enter_context(tc.tile_pool(name="v", bufs=3))
        y_pool = ctx.enter_context(tc.tile_pool(name="y", bufs=3))
        zt_pool = ctx.enter_context(tc.tile_pool(name="zt", bufs=4, space="PSUM"))
        po_pool = ctx.enter_context(tc.tile_pool(name="po", bufs=2, space="PSUM"))

        ident = const_pool.tile([128, 128], f32)
        nc.sync.dma_start(ident[:], i_d.ap()[:, :])
        hmat = const_pool.tile([128, 128], f32)
        nc.sync.dma_start(hmat[:], h_d.ap()[:, :])
        gvec = const_pool.tile([128, 8], f32)
        nc.sync.dma_start(gvec[:], g_d.ap()[:, :])

        for st in range(N_ST):
            r0 = st * ST_ROWS
            # ---- load supertile: [ST_ROWS, 1024] -> [128p, (rg f)] ----
            x_st = x_pool.tile([128, RG * DIM], f32)
            nc.sync.dma_start(
                x_st[:].rearrange("p (rg f) -> p rg f", rg=RG),
                x_d.ap()[r0:r0 + ST_ROWS, :].rearrange("(rg p) f -> p rg f", p=128),
            )

            # ---- transpose-in + golay-scaled PSUM drain ----
            u = u_pool.tile([128, RG * DIM], f32)
            for a in range(8):
                zt = zt_pool.tile([128, W], f32)
                for rg in range(RG):
                    nc.tensor.transpose(
                        zt[:, rg * 128:(rg + 1) * 128],
                        x_st[:, rg * DIM + a * 128: rg * DIM + (a + 1) * 128],
                        ident[:],
                    )
                # u_a[b, (rg r)] = zt_a[b, (rg r)] * g[a*128+b]
                nc.scalar.mul(u[:, a * W:(a + 1) * W], zt[:], gvec[:, a:a + 1])

            # ---- outer FHT_8: 3 butterfly levels over a (free-axis blocks of W) ----
            v = v_pool.tile([128, RG * DIM], f32)
            # level stride 4
            nc.vector.tensor_add(v[:, 0:4 * W], u[:, 0:4 * W], u[:, 4 * W:8 * W])
            nc.vector.tensor_sub(v[:, 4 * W:8 * W], u[:, 0:4 * W], u[:, 4 * W:8 * W])
            # level stride 2 (v -> u)
            u2 = u[:].rearrange("p (h q) -> p h q", h=2)
            v2 = v[:].rearrange("p (h q) -> p h q", h=2)
            nc.vector.tensor_add(u2[:, :, 0:2 * W], v2[:, :, 0:2 * W], v2[:, :, 2 * W:4 * W])
            nc.vector.tensor_sub(u2[:, :, 2 * W:4 * W], v2[:, :, 0:2 * W], v2[:, :, 2 * W:4 * W])
            # level stride 1 (u -> v)
            u4 = u[:].rearrange("p (q t) -> p q t", q=4)
            v4 = v[:].rearrange("p (q t) -> p q t", q=4)
            nc.vector.tensor_add(v4[:, :, 0:W], u4[:, :, 0:W], u4[:, :, W:2 * W])
            nc.vector.tensor_sub(v4[:, :, W:2 * W], u4[:, :, 0:W], u4[:, :, W:2 * W])

            # ---- inner H_128 matmul (un-transposes) + merged drain ----
            y_st = y_pool.tile([128, RG * DIM], f32)
            for rg in range(RG):
                po = po_pool.tile([128, DIM], f32)
                for a2 in range(8):
                    nc.tensor.matmul(
                        po[:, a2 * 128:(a2 + 1) * 128],
                        lhsT=v[:, a2 * W + rg * 128: a2 * W + (rg + 1) * 128],
                        rhs=hmat[:],
                        start=True, stop=True,
                    )
                nc.scalar.copy(y_st[:, rg * DIM:(rg + 1) * DIM], po[:])

            # ---- store ----
            nc.sync.dma_start(
                y_d.ap()[r0:r0 + ST_ROWS, :].rearrange("(rg p) f -> p rg f", p=128),
                y_st[:].rearrange("p (rg f) -> p rg f", rg=RG),
            )


_NC = None


def _get_nc():
    global _NC
    if _NC is None:
        _NC = _build_nc()
    return _NC


def kernel(x, golay, alpha):
    global LAST_RESULT
    x_np = np.ascontiguousarray(np.asarray(x, dtype=np.float32).reshape(ROWS_TOTAL, DIM))
    golay_np = np.asarray(golay, dtype=np.float32).reshape(DIM)
    alpha_np = np.float32(np.asarray(alpha, dtype=np.float32))

    s = np.float32(1.0) / (alpha_np + np.float32(EPS))
    gmat = np.ascontiguousarray((golay_np * s).reshape(8, 128).T)  # [b, a]
    hmat = _hadamard(128)
    ident = np.ascontiguousarray(np.eye(128, dtype=np.float32))

    nc = _get_nc()
    in_maps = [
        {
            "x": x_np[c * ROWS_PER_CORE:(c + 1) * ROWS_PER_CORE],
            "gvec": gmat,
            "hmat": hmat,
            "ident": ident,
        }
        for c in range(N_CORES)
    ]
    res = run_bass_kernel_spmd(nc, in_maps, core_ids=list(range(N_CORES)))
    LAST_RESULT = res
    y = np.concatenate([r["y"] for r in res.results], axis=0)
    return y.reshape(4, 8192, DIM)
